# revision 26
# baseline (speedup 1.0000x reference)
"""Trainium2 Bass kernel for nn_LoopyBeliefPropagation (B=8, S=128, 3 BP iters).

Math: the reference's loopy-BP collapses algebraically.  Writing m_sib in
terms of its q-difference dm (m0 = -softplus(dm), m1 = dm - softplus(dm),
exact after the per-edge logsumexp normalization) the update telescopes:

    dm1(i,j,k) = Db1(i,k) + softplus(sib(i,j,k)) - log2
    dm2(i,j,k) = Db2(i,k) - dm1(i,j,k) + softplus(sib) - log2
               = Db2(i,k) - Db1(i,k)            (j-independent!)

so the only use of the O(S^3) tensor is one masked-softplus row reduction

    C(i,j) = sum_k softplus(s_sib[b,j,i,k]) * mask[b,k,i]

and everything else is O(S^2) per batch:

    V(x,y)  = mask[b,x,y] (f32), Vt = V^T
    pe_q(i,j) = s_edge[b,j,i,q];  Dpe = pe1 - pe0
    Db1 = Dpe * V;  A(i) = sum_k Db1(i,k) Vt(i,k);  N(i) = sum_k Vt(i,k)
    Db2 = (Dpe + Vt * (A(i) + C - log2 * N(i))) * V
    E   = Db2 - Db1
    sP(i) = sum_k softplus(E(i,k)) Vt(i,k);  sE(i) = sum_k E(i,k) Vt(i,k)
    out[b,j,i,0] = (pe0(i,j) - Vt(i,j) sP(i)) * V(i,j)
    out[b,j,i,1] = (pe1(i,j) + Vt(i,j) (sE(i)-sP(i))) * V(i,j)

Phase-1 layout: mask[b,x,y] = valid[x]*valid[y] is rank-1 (sequence-length
masks), and downstream C is always multiplied by Vt(i,j) (which carries the
valid(i) factor), so the reduction only needs the k-mask:

    C'(i,j) = sum_k softplus(ss[j,i,k]) * valid(k),   (C'+G)*Vt == (C+G)*Vt

valid(k) = mask[1,k] (index 1 is always valid: lens >= S/2 > 1).  This lets
the big tensor stream with partition=j in its NATIVE layout (each partition
reads one contiguous 16KB block per chunk -- optimal DMA descriptors), with
the k-mask broadcast along free axes.

There is no usable softplus ACT table (the pwp softplus slot is the opaque
'act2'), so the masked softplus sum is evaluated in product space with ONE
big Exp pass instead of two (Exp + Ln) passes:

    sum_k ln(1+e^x_k)*m_k = sum_{groups g of 8} ln( prod_{k in g} t_k ),
    t_k = min(1 + e^x_k, M_k),  M_k = +BIG if valid(k) else 1

(t_k == 1+e^x_k when valid since 1+e < BIG; == 1 when masked since 1+e >= 1;
group size 8 keeps prod <= (1+e^6)^8 ~ 1e20 well inside f32/bf16 range).

Engine assignment is driven by the DVE perf-mode table (scalar_tensor_tensor
and tensor_reduce NEVER pack -> 1 elem/cycle; tensor_tensor packs 2x for
bf16; tensor_scalar/copy pack 4x): the fused add-min runs as ONE gpsimd
scalar_tensor_tensor (otherwise-idle engine), the group product is a 3-level
pairwise tensor_tensor multiply tree on DVE (2x mode), and Ln runs on S*16
elements per chunk instead of S*S.  The result Ct[j,i] is transposed on the
(idle) TensorE at the end.

Sharding: data-parallel over batch, one batch per NeuronCore (8 cores).
"""

import numpy as np

import concourse.bass as bass
import concourse.bacc as bacc
import concourse.tile as tile
from concourse import mybir
from concourse.bass_utils import run_bass_kernel_spmd
from concourse.masks import make_identity

B, S = 8, 128
LOG2 = float(np.log(2.0))
FP32 = mybir.dt.float32
BF16 = mybir.dt.bfloat16
AF = mybir.ActivationFunctionType
OP = mybir.AluOpType

# i-slabs per DMA chunk in the big s_sib loop; product-group size
GI = 16
PG = 8
BIG = 1.0e38


def _pin_act_tables():
    """Restrict activation-table choice to natural_log_exp_and_others (which
    holds every ACT func this kernel uses: exp, ln, abs, relu) so Bacc's
    table-load pass never switches sets (~1.3us per reload).  Set ids are
    positional, so other entries are emptied rather than removed."""
    import concourse.hw_specs as hw_specs

    if getattr(hw_specs.get_activation_tables, "_bp_pinned", False):
        return
    orig = hw_specs.get_activation_tables

    def pinned(module_arch):
        tables = orig(module_arch)
        return {
            name: (funcs if name == "natural_log_exp_and_others" else set())
            for name, funcs in tables.items()
        }

    pinned._bp_pinned = True
    hw_specs.get_activation_tables = pinned
    import concourse.bacc as _bacc_mod

    if getattr(_bacc_mod, "get_activation_tables", None) is orig:
        _bacc_mod.get_activation_tables = pinned


def build_kernel_module(reps: int = 1, loop_n: int = 0):
    _pin_act_tables()
    nc = bacc.Bacc("TRN2", debug=False, target_bir_lowering=False)

    ss = nc.dram_tensor("ss", [S, S, S], FP32, kind="ExternalInput")   # s_sib[b]  (j,i,k)
    se = nc.dram_tensor("se", [S, 2 * S], FP32, kind="ExternalInput")  # s_edge[b] (j, i*2+q)
    mk = nc.dram_tensor("mk", [S, S], FP32, kind="ExternalInput")      # mask[b] as f32
    out = nc.dram_tensor("out", [S, 2 * S], FP32, kind="ExternalOutput")

    with tile.TileContext(nc) as tc:
        with (
            tc.tile_pool(name="consts", bufs=1) as consts,
            tc.tile_pool(name="small", bufs=2) as small,
            tc.tile_pool(name="chunks", bufs=4) as chunks,
            tc.tile_pool(name="spp", bufs=4) as spp,
            tc.tile_pool(name="tpp", bufs=4) as tpp,
            tc.tile_pool(name="mp1", bufs=2) as mp1,
            tc.tile_pool(name="mp2", bufs=2) as mp2,
            tc.tile_pool(name="mp3", bufs=3) as mp3,
            tc.tile_pool(name="lpp", bufs=3) as lpp,
            tc.tile_pool(name="scratch", bufs=2) as scratch,
            tc.tile_pool(name="psum", bufs=1, space="PSUM") as psum,
        ):
          # chunk i-slab sizes: small edges shorten pipeline fill/drain
          SIZES = [4, 8, 8] + [16] * 6 + [8, 4]
          OFFS = [sum(SIZES[:c]) for c in range(len(SIZES))]
          # middle (large) chunks mask on gpsimd; edge chunks mask on DVE so
          # neither the startup nor the drain waits on the gpsimd backlog

          def _body():
                # ---- phase 0 + phase 1 interleaved ---------------------------
                # DMA issue order matters (single SP queue): chunk 0 goes first
                # so compute starts ASAP; se (only needed by the finale) goes
                # after the last chunk.
                ident = consts.tile([S, S], FP32)
                make_identity(nc, ident)

                nch = len(SIZES)
                nxt = chunks.tile([S, GI, S], FP32, name="chunk")
                nc.sync.dma_start(
                    out=nxt[:, : SIZES[0], :], in_=ss[:, : SIZES[0], :]
                )

                # vk[p,k] = mask[1,k] = valid(k), replicated to all partitions
                # by a stride-0 broadcast DMA of DRAM row 1 (index 1 is always
                # valid: lens >= S/2 > 1).  Turned into the min-mask
                # M = valid ? BIG : 1 so that t = min(1+e, M) fuses the +1 and
                # the masking into one op.
                vkf = consts.tile([S, S], FP32)
                nc.sync.dma_start(out=vkf, in_=mk[1:2, :].to_broadcast([S, S]))
                V = consts.tile([S, S], FP32)
                nc.sync.dma_start(out=V, in_=mk[:])

                vkm = consts.tile([S, S], BF16)
                nc.vector.tensor_scalar(
                    out=vkm[:], in0=vkf[:], scalar1=BIG, scalar2=1.0,
                    op0=OP.mult, op1=OP.add,
                )
                # replicate along the i-slab axis once so the per-chunk
                # fused add-min runs with unit strides
                vkrep = consts.tile([S, GI, S], BF16)
                nc.vector.tensor_copy(
                    vkrep[:], vkm[:, None, :].broadcast_to([S, GI, S])
                )

                # mask is symmetric rank-1 (mask[x,y]=valid[x]*valid[y]), so
                # Vt == V and V*V == V; Db1*Vt == Dpe*V, sums collapse.
                stats = consts.tile([S, 8], FP32)  # cols: A, N, G, sP, sE, sD, nsP

                # phase 1: Ct(j,i) = sum_k softplus(ss[j,i,k]) * valid(k)
                # native-layout DMA (partition=j, one contiguous block per
                # partition per chunk); product-space masked softplus:
                # Exp (ACT) -> t=min(1+e, M) (one fused gpsimd op) ->
                # pairwise multiply tree to products of 8 (DVE 2x) ->
                # Ln on S*gi*16 (ACT) -> group add-reduce (DVE).
                Ct = consts.tile([S, S], FP32)
                NGRP = S // PG

                def _ln_and_reduce(c, m3):
                    gi, i0 = SIZES[c], OFFS[c]
                    lnb = lpp.tile([S, GI, NGRP], FP32, name="lnb")
                    nc.scalar.activation(lnb[:, :gi, :], m3[:, :gi, :], AF.Ln)
                    nc.vector.tensor_reduce(
                        out=Ct[:, i0:i0 + gi], in_=lnb[:, :gi, :],
                        axis=mybir.AxisListType.X, op=OP.add,
                    )

                # issue order is software-pipelined: Exp(c) is queued on ACT
                # BEFORE Ln(c-1) so a stalled Ln never delays the next chunk's
                # Exp in the in-order engine queue.
                #
                # lens >= S/2, so k in [1, S/2) is ALWAYS valid and only the
                # high half k in [S/2, S) (plus the always-invalid k=0 column,
                # zeroed at compile time) needs the data-dependent min-mask:
                # the +1 (a legal Pool tensor_scalar) runs on gpsimd for the
                # large middle chunks, the half-width min runs on DVE.
                H = S // 2
                pend = None  # (c, m3) awaiting Ln+reduce
                for c in range(nch):
                    gi, i0 = SIZES[c], OFFS[c]
                    chunk = nxt
                    if c + 1 < nch:
                        nxt = chunks.tile([S, GI, S], FP32, name="chunk")
                        nc.sync.dma_start(
                            out=nxt[:, : SIZES[c + 1], :],
                            in_=ss[:, OFFS[c + 1] : OFFS[c + 1] + SIZES[c + 1], :],
                        )
                    eb = spp.tile([S, GI, S], BF16)
                    nc.scalar.activation(
                        eb[:, :gi, :], chunk[:, :gi, :], AF.Exp
                    )
                    if pend is not None:
                        _ln_and_reduce(*pend)
                    tb = tpp.tile([S, GI, S], BF16)
                    if 3 <= c <= 8:
                        nc.gpsimd.tensor_scalar(
                            out=tb[:, :gi, :], in0=eb[:, :gi, :], scalar1=1.0,
                            scalar2=None, op0=OP.add,
                        )
                    else:
                        nc.vector.tensor_scalar(
                            out=tb[:, :gi, :], in0=eb[:, :gi, :], scalar1=1.0,
                            scalar2=None, op0=OP.add,
                        )
                    # data-dependent mask on the high half of k, plus the
                    # always-invalid k=0 column (u0 >= 1 so min(u0,1)==1)
                    nc.vector.tensor_tensor(
                        tb[:, :gi, 0:1], tb[:, :gi, 0:1], vkrep[:, :gi, 0:1],
                        OP.min,
                    )
                    nc.vector.tensor_tensor(
                        tb[:, :gi, H:], tb[:, :gi, H:], vkrep[:, :gi, H:],
                        OP.min,
                    )
                    m1 = mp1.tile([S, GI, 64], BF16)
                    nc.vector.tensor_tensor(
                        m1[:, :gi, :], tb[:, :gi, 0:64], tb[:, :gi, 64:128],
                        OP.mult,
                    )
                    m2 = mp2.tile([S, GI, 32], BF16)
                    nc.vector.tensor_tensor(
                        m2[:, :gi, :], m1[:, :gi, 0:32], m1[:, :gi, 32:64],
                        OP.mult,
                    )
                    m3 = mp3.tile([S, GI, 16], BF16)
                    nc.vector.tensor_tensor(
                        m3[:, :gi, :], m2[:, :gi, 0:16], m2[:, :gi, 16:32],
                        OP.mult,
                    )
                    pend = (c, m3)
                _ln_and_reduce(*pend)

                # se DMA + small prep, issued after the chunk stream
                se_sb = small.tile([S, 2 * S], FP32)
                nc.sync.dma_start(out=se_sb, in_=se[:])
                se3 = se_sb[:].rearrange("p (i q) -> p i q", q=2)

                pe0_ps = psum.tile([S, S], FP32, tag="pe0_ps")
                nc.tensor.transpose(pe0_ps[:], se3[:, :, 0], ident[:])
                pe0 = consts.tile([S, S], FP32)
                nc.vector.tensor_copy(pe0[:], pe0_ps[:])

                pe1_ps = psum.tile([S, S], FP32, tag="pe1_ps")
                nc.tensor.transpose(pe1_ps[:], se3[:, :, 1], ident[:])
                pe1 = consts.tile([S, S], FP32)
                nc.vector.tensor_copy(pe1[:], pe1_ps[:])

                Dpe = consts.tile([S, S], FP32)
                nc.vector.tensor_tensor(Dpe[:], pe1[:], pe0[:], OP.subtract)

                # A = sum_k Dpe*V ; N = sum_k V ; G = A - log2 * N
                scr0 = scratch.tile([S, S], FP32)
                nc.vector.scalar_tensor_tensor(
                    out=scr0[:], in0=Dpe[:], scalar=1.0, in1=V[:],
                    op0=OP.mult, op1=OP.mult, accum_out=stats[:, 0:1],
                )
                nc.vector.tensor_reduce(
                    out=stats[:, 1:2], in_=V[:], axis=mybir.AxisListType.X, op=OP.add,
                )
                nc.vector.scalar_tensor_tensor(
                    out=stats[:, 2:3], in0=stats[:, 1:2], scalar=-LOG2,
                    in1=stats[:, 0:1], op0=OP.mult, op1=OP.add,
                )

                ct_ps = psum.tile([S, S], FP32, tag="ct_ps")
                nc.tensor.transpose(ct_ps[:], Ct[:], ident[:])

                # ---- phase 2: finale -----------------------------------------
                # E = Db2 - Db1 = (C + G) * V  (exact under mask symmetry)
                E = small.tile([S, S], FP32)
                nc.vector.scalar_tensor_tensor(
                    out=E[:], in0=ct_ps[:], scalar=stats[:, 2:3], in1=V[:],
                    op0=OP.add, op1=OP.mult,
                )

                # stable softplus row sums via ACT accumulators.  E is already
                # masked, so sum relu(E)*V == sum relu(E); the ln1p term is
                # summed UNMASKED (masked entries contribute ln2 each) and
                # corrected by -(S-N)*ln2:
                #   sP = sum relu(E) + sum Ln(1+Exp(-|E|)) - (S-N)*log2
                aE = small.tile([S, S], FP32)
                nc.scalar.activation(aE[:], E[:], AF.Abs)
                nc.scalar.activation(aE[:], aE[:], AF.Exp, scale=-1.0)
                lnp = scratch.tile([S, S], FP32)
                nc.scalar.activation(
                    lnp[:], aE[:], AF.Ln, bias=1.0, accum_out=stats[:, 3:4]
                )
                rel = scratch.tile([S, S], FP32)
                nc.scalar.activation(
                    rel[:], E[:], AF.Relu, accum_out=stats[:, 7:8]
                )
                nc.vector.tensor_reduce(
                    out=stats[:, 4:5], in_=E[:], axis=mybir.AxisListType.X, op=OP.add,
                )
                # sP = (sLn + sRelu) + log2*N - S*log2
                nc.vector.tensor_tensor(
                    stats[:, 3:4], stats[:, 3:4], stats[:, 7:8], OP.add
                )
                nc.vector.scalar_tensor_tensor(
                    out=stats[:, 3:4], in0=stats[:, 1:2], scalar=LOG2,
                    in1=stats[:, 3:4], op0=OP.mult, op1=OP.add,
                )
                nc.vector.tensor_scalar(
                    out=stats[:, 3:4], in0=stats[:, 3:4], scalar1=-S * LOG2,
                    scalar2=None, op0=OP.add,
                )
                # sD = sE - sP ; nsP = -sP
                nc.vector.tensor_tensor(
                    stats[:, 5:6], stats[:, 4:5], stats[:, 3:4], OP.subtract
                )
                nc.vector.tensor_scalar(
                    out=stats[:, 6:7], in0=stats[:, 3:4], scalar1=-1.0, scalar2=None,
                    op0=OP.mult,
                )

                # b3_0 = (pe0 - sP) * V ; b3_1 = (pe1 + sD) * V  (V*V == V)
                b30 = small.tile([S, S], FP32)
                nc.vector.scalar_tensor_tensor(
                    out=b30[:], in0=pe0[:], scalar=stats[:, 6:7], in1=V[:],
                    op0=OP.add, op1=OP.mult,
                )
                b31 = small.tile([S, S], FP32)
                nc.vector.scalar_tensor_tensor(
                    out=b31[:], in0=pe1[:], scalar=stats[:, 5:6], in1=V[:],
                    op0=OP.add, op1=OP.mult,
                )

                t0_ps = psum.tile([S, S], FP32, tag="t0_ps")
                nc.tensor.transpose(t0_ps[:], b30[:], ident[:])
                t1_ps = psum.tile([S, S], FP32, tag="t1_ps")
                nc.tensor.transpose(t1_ps[:], b31[:], ident[:])

                outT = small.tile([S, 2 * S], FP32)
                out3 = outT[:].rearrange("p (i q) -> p i q", q=2)
                nc.vector.tensor_copy(out3[:, :, 0], t0_ps[:])
                nc.vector.tensor_copy(out3[:, :, 1], t1_ps[:])
                nc.sync.dma_start(out=out[:], in_=outT)

          if loop_n > 1:
              with tc.For_i(0, loop_n, 1):
                  _body()
          else:
              for _rep in range(reps):
                  _body()

    nc.compile()
    return nc


_NC_CACHE = None


def _get_nc():
    global _NC_CACHE
    if _NC_CACHE is None:
        _NC_CACHE = build_kernel_module()
    return _NC_CACHE


def kernel(s_edge: np.ndarray, s_sib: np.ndarray, mask: np.ndarray) -> np.ndarray:
    s_edge = np.ascontiguousarray(np.asarray(s_edge, dtype=np.float32))
    s_sib = np.ascontiguousarray(np.asarray(s_sib, dtype=np.float32))
    mask_f = np.ascontiguousarray(np.asarray(mask).astype(np.float32))

    nc = _get_nc()
    in_maps = [
        {
            "ss": s_sib[b],
            "se": s_edge[b].reshape(S, 2 * S),
            "mk": mask_f[b],
        }
        for b in range(B)
    ]
    res = run_bass_kernel_spmd(nc, in_maps, core_ids=list(range(B)))
    out = np.stack([res.results[b]["out"].reshape(S, S, 2) for b in range(B)])
    return out.astype(np.float32)


if __name__ == "__main__":
    rng = np.random.default_rng(0)
    se_ = rng.standard_normal((B, S, S, 2), dtype=np.float32)
    sib_ = rng.standard_normal((B, S, S, S), dtype=np.float32)
    mk_ = np.ones((B, S, S), dtype=bool)
    print(kernel(se_, sib_, mk_).shape)


# revision 27
# speedup vs baseline: 5.8883x; 5.8883x over previous
"""Trainium2 Bass kernel for nn_LoopyBeliefPropagation (B=8, S=128, 3 BP iters).

Math: the reference's loopy-BP collapses algebraically.  Writing m_sib in
terms of its q-difference dm (m0 = -softplus(dm), m1 = dm - softplus(dm),
exact after the per-edge logsumexp normalization) the update telescopes:

    dm1(i,j,k) = Db1(i,k) + softplus(sib(i,j,k)) - log2
    dm2(i,j,k) = Db2(i,k) - dm1(i,j,k) + softplus(sib) - log2
               = Db2(i,k) - Db1(i,k)            (j-independent!)

so the only use of the O(S^3) tensor is one masked-softplus row reduction

    C(i,j) = sum_k softplus(s_sib[b,j,i,k]) * mask[b,k,i]

and everything else is O(S^2) per batch:

    V(x,y)  = mask[b,x,y] (f32), Vt = V^T
    pe_q(i,j) = s_edge[b,j,i,q];  Dpe = pe1 - pe0
    Db1 = Dpe * V;  A(i) = sum_k Db1(i,k) Vt(i,k);  N(i) = sum_k Vt(i,k)
    Db2 = (Dpe + Vt * (A(i) + C - log2 * N(i))) * V
    E   = Db2 - Db1
    sP(i) = sum_k softplus(E(i,k)) Vt(i,k);  sE(i) = sum_k E(i,k) Vt(i,k)
    out[b,j,i,0] = (pe0(i,j) - Vt(i,j) sP(i)) * V(i,j)
    out[b,j,i,1] = (pe1(i,j) + Vt(i,j) (sE(i)-sP(i))) * V(i,j)

Phase-1 layout: mask[b,x,y] = valid[x]*valid[y] is rank-1 (sequence-length
masks), and downstream C is always multiplied by Vt(i,j) (which carries the
valid(i) factor), so the reduction only needs the k-mask:

    C'(i,j) = sum_k softplus(ss[j,i,k]) * valid(k),   (C'+G)*Vt == (C+G)*Vt

valid(k) = mask[1,k] (index 1 is always valid: lens >= S/2 > 1).  This lets
the big tensor stream with partition=j in its NATIVE layout (each partition
reads one contiguous 16KB block per chunk -- optimal DMA descriptors), with
the k-mask broadcast along free axes.

There is no usable softplus ACT table (the pwp softplus slot is the opaque
'act2'), so the masked softplus sum is evaluated in product space with ONE
big Exp pass instead of two (Exp + Ln) passes:

    sum_k ln(1+e^x_k)*m_k = sum_{groups g of 8} ln( prod_{k in g} t_k ),
    t_k = min(1 + e^x_k, M_k),  M_k = +BIG if valid(k) else 1

(t_k == 1+e^x_k when valid since 1+e < BIG; == 1 when masked since 1+e >= 1;
group size 8 keeps prod <= (1+e^6)^8 ~ 1e20 well inside f32/bf16 range).

Engine assignment is driven by the DVE perf-mode table (scalar_tensor_tensor
and tensor_reduce NEVER pack -> 1 elem/cycle; tensor_tensor packs 2x for
bf16; tensor_scalar/copy pack 4x): the fused add-min runs as ONE gpsimd
scalar_tensor_tensor (otherwise-idle engine), the group product is a 3-level
pairwise tensor_tensor multiply tree on DVE (2x mode), and Ln runs on S*16
elements per chunk instead of S*S.  The result Ct[j,i] is transposed on the
(idle) TensorE at the end.

Sharding: data-parallel over batch, one batch per NeuronCore (8 cores).
"""

import numpy as np

import concourse.bass as bass
import concourse.bacc as bacc
import concourse.tile as tile
from concourse import mybir
from concourse.bass_utils import run_bass_kernel_spmd
from concourse.masks import make_identity

B, S = 8, 128
LOG2 = float(np.log(2.0))
FP32 = mybir.dt.float32
BF16 = mybir.dt.bfloat16
AF = mybir.ActivationFunctionType
OP = mybir.AluOpType

# i-slabs per DMA chunk in the big s_sib loop; product-group size
GI = 16
PG = 8
BIG = 1.0e38


def _pin_act_tables():
    """Restrict activation-table choice to natural_log_exp_and_others (which
    holds every ACT func this kernel uses: exp, ln, abs, relu) so Bacc's
    table-load pass never switches sets (~1.3us per reload).  Set ids are
    positional, so other entries are emptied rather than removed."""
    import concourse.hw_specs as hw_specs

    if getattr(hw_specs.get_activation_tables, "_bp_pinned", False):
        return
    orig = hw_specs.get_activation_tables

    def pinned(module_arch):
        tables = orig(module_arch)
        return {
            name: (funcs if name == "natural_log_exp_and_others" else set())
            for name, funcs in tables.items()
        }

    pinned._bp_pinned = True
    hw_specs.get_activation_tables = pinned
    import concourse.bacc as _bacc_mod

    if getattr(_bacc_mod, "get_activation_tables", None) is orig:
        _bacc_mod.get_activation_tables = pinned


def build_kernel_module(reps: int = 1, loop_n: int = 0):
    _pin_act_tables()
    nc = bacc.Bacc("TRN2", debug=False, target_bir_lowering=False)

    ss = nc.dram_tensor("ss", [S, S, S], FP32, kind="ExternalInput")   # s_sib[b]  (j,i,k)
    se = nc.dram_tensor("se", [S, 2 * S], FP32, kind="ExternalInput")  # s_edge[b] (j, i*2+q)
    mk = nc.dram_tensor("mk", [S, S], FP32, kind="ExternalInput")      # mask[b] as f32
    out = nc.dram_tensor("out", [S, 2 * S], FP32, kind="ExternalOutput")

    with tile.TileContext(nc) as tc:
        with (
            tc.tile_pool(name="consts", bufs=1) as consts,
            tc.tile_pool(name="small", bufs=2) as small,
            tc.tile_pool(name="chunks", bufs=4) as chunks,
            tc.tile_pool(name="spp", bufs=4) as spp,
            tc.tile_pool(name="tpp", bufs=4) as tpp,
            tc.tile_pool(name="mp1", bufs=2) as mp1,
            tc.tile_pool(name="mp2", bufs=2) as mp2,
            tc.tile_pool(name="mp3", bufs=3) as mp3,
            tc.tile_pool(name="lpp", bufs=3) as lpp,
            tc.tile_pool(name="scratch", bufs=2) as scratch,
            tc.tile_pool(name="psum", bufs=1, space="PSUM") as psum,
        ):
          # chunk i-slab sizes: small edges shorten pipeline fill/drain
          SIZES = [4, 8, 8] + [16] * 6 + [8, 4]
          OFFS = [sum(SIZES[:c]) for c in range(len(SIZES))]
          # middle (large) chunks mask on gpsimd; edge chunks mask on DVE so
          # neither the startup nor the drain waits on the gpsimd backlog

          def _body():
                # ---- phase 0 + phase 1 interleaved ---------------------------
                # DMA issue order matters (single SP queue): chunk 0 goes first
                # so compute starts ASAP; se (only needed by the finale) goes
                # after the last chunk.
                ident = consts.tile([S, S], FP32)
                make_identity(nc, ident)

                nch = len(SIZES)
                nxt = chunks.tile([S, GI, S], FP32, name="chunk")
                nc.sync.dma_start(
                    out=nxt[:, : SIZES[0], :], in_=ss[:, : SIZES[0], :]
                )

                # vk[p,k] = mask[1,k] = valid(k), replicated to all partitions
                # by a stride-0 broadcast DMA of DRAM row 1 (index 1 is always
                # valid: lens >= S/2 > 1).  Turned into the min-mask
                # M = valid ? BIG : 1 so that t = min(1+e, M) fuses the +1 and
                # the masking into one op.
                vkf = consts.tile([S, S], FP32)
                nc.sync.dma_start(out=vkf, in_=mk[1:2, :].to_broadcast([S, S]))
                V = consts.tile([S, S], FP32)
                nc.sync.dma_start(out=V, in_=mk[:])

                vkm = consts.tile([S, S], BF16)
                nc.vector.tensor_scalar(
                    out=vkm[:], in0=vkf[:], scalar1=BIG, scalar2=1.0,
                    op0=OP.mult, op1=OP.add,
                )
                # replicate along the i-slab axis once so the per-chunk
                # fused add-min runs with unit strides
                vkrep = consts.tile([S, GI, S], BF16)
                nc.vector.tensor_copy(
                    vkrep[:], vkm[:, None, :].broadcast_to([S, GI, S])
                )

                # mask is symmetric rank-1 (mask[x,y]=valid[x]*valid[y]), so
                # Vt == V and V*V == V; Db1*Vt == Dpe*V, sums collapse.
                stats = consts.tile([S, 8], FP32)  # cols: A, N, G, sP, sE, sD, nsP

                # phase 1: Ct(j,i) = sum_k softplus(ss[j,i,k]) * valid(k)
                # native-layout DMA (partition=j, one contiguous block per
                # partition per chunk); product-space masked softplus:
                # Exp (ACT) -> t=min(1+e, M) (one fused gpsimd op) ->
                # pairwise multiply tree to products of 8 (DVE 2x) ->
                # Ln on S*gi*16 (ACT) -> group add-reduce (DVE).
                Ct = consts.tile([S, S], FP32)
                NGRP = S // PG

                def _ln_and_reduce(c, m3):
                    gi, i0 = SIZES[c], OFFS[c]
                    lnb = lpp.tile([S, GI, NGRP], FP32, name="lnb")
                    nc.scalar.activation(lnb[:, :gi, :], m3[:, :gi, :], AF.Ln)
                    nc.vector.tensor_reduce(
                        out=Ct[:, i0:i0 + gi], in_=lnb[:, :gi, :],
                        axis=mybir.AxisListType.X, op=OP.add,
                    )

                # issue order is software-pipelined: Exp(c) is queued on ACT
                # BEFORE Ln(c-1) so a stalled Ln never delays the next chunk's
                # Exp in the in-order engine queue.
                #
                # lens >= S/2, so k in [1, S/2) is ALWAYS valid and only the
                # high half k in [S/2, S) (plus the always-invalid k=0 column,
                # zeroed at compile time) needs the data-dependent min-mask:
                # the +1 (a legal Pool tensor_scalar) runs on gpsimd for the
                # large middle chunks, the half-width min runs on DVE.
                H = S // 2
                pend = None  # (c, m3) awaiting Ln+reduce
                for c in range(nch):
                    gi, i0 = SIZES[c], OFFS[c]
                    chunk = nxt
                    if c + 1 < nch:
                        nxt = chunks.tile([S, GI, S], FP32, name="chunk")
                        nc.sync.dma_start(
                            out=nxt[:, : SIZES[c + 1], :],
                            in_=ss[:, OFFS[c + 1] : OFFS[c + 1] + SIZES[c + 1], :],
                        )
                    eb = spp.tile([S, GI, S], BF16)
                    nc.scalar.activation(
                        eb[:, :gi, :], chunk[:, :gi, :], AF.Exp
                    )
                    if pend is not None:
                        _ln_and_reduce(*pend)
                    tb = tpp.tile([S, GI, S], BF16)
                    nc.vector.tensor_scalar(
                        out=tb[:, :gi, :], in0=eb[:, :gi, :], scalar1=1.0,
                        scalar2=None, op0=OP.add,
                    )
                    # data-dependent mask on the high half of k, plus the
                    # always-invalid k=0 column (u0 >= 1 so min(u0,1)==1)
                    nc.vector.tensor_tensor(
                        tb[:, :gi, 0:1], tb[:, :gi, 0:1], vkrep[:, :gi, 0:1],
                        OP.min,
                    )
                    nc.vector.tensor_tensor(
                        tb[:, :gi, H:], tb[:, :gi, H:], vkrep[:, :gi, H:],
                        OP.min,
                    )
                    m1 = mp1.tile([S, GI, 64], BF16)
                    nc.vector.tensor_tensor(
                        m1[:, :gi, :], tb[:, :gi, 0:64], tb[:, :gi, 64:128],
                        OP.mult,
                    )
                    m2 = mp2.tile([S, GI, 32], BF16)
                    nc.vector.tensor_tensor(
                        m2[:, :gi, :], m1[:, :gi, 0:32], m1[:, :gi, 32:64],
                        OP.mult,
                    )
                    m3 = mp3.tile([S, GI, 16], BF16)
                    nc.vector.tensor_tensor(
                        m3[:, :gi, :], m2[:, :gi, 0:16], m2[:, :gi, 16:32],
                        OP.mult,
                    )
                    pend = (c, m3)
                _ln_and_reduce(*pend)

                # se DMA + small prep, issued after the chunk stream
                se_sb = small.tile([S, 2 * S], FP32)
                nc.sync.dma_start(out=se_sb, in_=se[:])
                se3 = se_sb[:].rearrange("p (i q) -> p i q", q=2)

                pe0_ps = psum.tile([S, S], FP32, tag="pe0_ps")
                nc.tensor.transpose(pe0_ps[:], se3[:, :, 0], ident[:])
                pe0 = consts.tile([S, S], FP32)
                nc.vector.tensor_copy(pe0[:], pe0_ps[:])

                pe1_ps = psum.tile([S, S], FP32, tag="pe1_ps")
                nc.tensor.transpose(pe1_ps[:], se3[:, :, 1], ident[:])
                pe1 = consts.tile([S, S], FP32)
                nc.vector.tensor_copy(pe1[:], pe1_ps[:])

                Dpe = consts.tile([S, S], FP32)
                nc.vector.tensor_tensor(Dpe[:], pe1[:], pe0[:], OP.subtract)

                # A = sum_k Dpe*V ; N = sum_k V ; G = A - log2 * N
                scr0 = scratch.tile([S, S], FP32)
                nc.vector.scalar_tensor_tensor(
                    out=scr0[:], in0=Dpe[:], scalar=1.0, in1=V[:],
                    op0=OP.mult, op1=OP.mult, accum_out=stats[:, 0:1],
                )
                nc.vector.tensor_reduce(
                    out=stats[:, 1:2], in_=V[:], axis=mybir.AxisListType.X, op=OP.add,
                )
                nc.vector.scalar_tensor_tensor(
                    out=stats[:, 2:3], in0=stats[:, 1:2], scalar=-LOG2,
                    in1=stats[:, 0:1], op0=OP.mult, op1=OP.add,
                )

                ct_ps = psum.tile([S, S], FP32, tag="ct_ps")
                nc.tensor.transpose(ct_ps[:], Ct[:], ident[:])

                # ---- phase 2: finale -----------------------------------------
                # E = Db2 - Db1 = (C + G) * V  (exact under mask symmetry)
                E = small.tile([S, S], FP32)
                nc.vector.scalar_tensor_tensor(
                    out=E[:], in0=ct_ps[:], scalar=stats[:, 2:3], in1=V[:],
                    op0=OP.add, op1=OP.mult,
                )

                # stable softplus row sums via ACT accumulators.  E is already
                # masked, so sum relu(E)*V == sum relu(E); the ln1p term is
                # summed UNMASKED (masked entries contribute ln2 each) and
                # corrected by -(S-N)*ln2:
                #   sP = sum relu(E) + sum Ln(1+Exp(-|E|)) - (S-N)*log2
                aE = small.tile([S, S], FP32)
                nc.scalar.activation(aE[:], E[:], AF.Abs)
                nc.scalar.activation(aE[:], aE[:], AF.Exp, scale=-1.0)
                lnp = scratch.tile([S, S], FP32)
                nc.scalar.activation(
                    lnp[:], aE[:], AF.Ln, bias=1.0, accum_out=stats[:, 3:4]
                )
                rel = scratch.tile([S, S], FP32)
                nc.scalar.activation(
                    rel[:], E[:], AF.Relu, accum_out=stats[:, 7:8]
                )
                nc.vector.tensor_reduce(
                    out=stats[:, 4:5], in_=E[:], axis=mybir.AxisListType.X, op=OP.add,
                )
                # sP = (sLn + sRelu) + log2*N - S*log2
                nc.vector.tensor_tensor(
                    stats[:, 3:4], stats[:, 3:4], stats[:, 7:8], OP.add
                )
                nc.vector.scalar_tensor_tensor(
                    out=stats[:, 3:4], in0=stats[:, 1:2], scalar=LOG2,
                    in1=stats[:, 3:4], op0=OP.mult, op1=OP.add,
                )
                nc.vector.tensor_scalar(
                    out=stats[:, 3:4], in0=stats[:, 3:4], scalar1=-S * LOG2,
                    scalar2=None, op0=OP.add,
                )
                # sD = sE - sP ; nsP = -sP
                nc.vector.tensor_tensor(
                    stats[:, 5:6], stats[:, 4:5], stats[:, 3:4], OP.subtract
                )
                nc.vector.tensor_scalar(
                    out=stats[:, 6:7], in0=stats[:, 3:4], scalar1=-1.0, scalar2=None,
                    op0=OP.mult,
                )

                # b3_0 = (pe0 - sP) * V ; b3_1 = (pe1 + sD) * V  (V*V == V)
                b30 = small.tile([S, S], FP32)
                nc.vector.scalar_tensor_tensor(
                    out=b30[:], in0=pe0[:], scalar=stats[:, 6:7], in1=V[:],
                    op0=OP.add, op1=OP.mult,
                )
                b31 = small.tile([S, S], FP32)
                nc.vector.scalar_tensor_tensor(
                    out=b31[:], in0=pe1[:], scalar=stats[:, 5:6], in1=V[:],
                    op0=OP.add, op1=OP.mult,
                )

                t0_ps = psum.tile([S, S], FP32, tag="t0_ps")
                nc.tensor.transpose(t0_ps[:], b30[:], ident[:])
                t1_ps = psum.tile([S, S], FP32, tag="t1_ps")
                nc.tensor.transpose(t1_ps[:], b31[:], ident[:])

                outT = small.tile([S, 2 * S], FP32)
                out3 = outT[:].rearrange("p (i q) -> p i q", q=2)
                nc.vector.tensor_copy(out3[:, :, 0], t0_ps[:])
                nc.vector.tensor_copy(out3[:, :, 1], t1_ps[:])
                nc.sync.dma_start(out=out[:], in_=outT)

          if loop_n > 1:
              with tc.For_i(0, loop_n, 1):
                  _body()
          else:
              for _rep in range(reps):
                  _body()

    nc.compile()
    return nc


_NC_CACHE = None


def _get_nc():
    global _NC_CACHE
    if _NC_CACHE is None:
        _NC_CACHE = build_kernel_module()
    return _NC_CACHE


def kernel(s_edge: np.ndarray, s_sib: np.ndarray, mask: np.ndarray) -> np.ndarray:
    s_edge = np.ascontiguousarray(np.asarray(s_edge, dtype=np.float32))
    s_sib = np.ascontiguousarray(np.asarray(s_sib, dtype=np.float32))
    mask_f = np.ascontiguousarray(np.asarray(mask).astype(np.float32))

    nc = _get_nc()
    in_maps = [
        {
            "ss": s_sib[b],
            "se": s_edge[b].reshape(S, 2 * S),
            "mk": mask_f[b],
        }
        for b in range(B)
    ]
    res = run_bass_kernel_spmd(nc, in_maps, core_ids=list(range(B)))
    out = np.stack([res.results[b]["out"].reshape(S, S, 2) for b in range(B)])
    return out.astype(np.float32)


if __name__ == "__main__":
    rng = np.random.default_rng(0)
    se_ = rng.standard_normal((B, S, S, 2), dtype=np.float32)
    sib_ = rng.standard_normal((B, S, S, S), dtype=np.float32)
    mk_ = np.ones((B, S, S), dtype=bool)
    print(kernel(se_, sib_, mk_).shape)
